# revision 26
# baseline (speedup 1.0000x reference)
"""DSNT double-loss kernel for Trainium2 (8 NeuronCores, batch-sharded).

Problem: input/target [32, 8, 256, 256] f32.
Per (b,c) pair: softmax-DSNT expected coords from `input`, argmax coords
from `target`, euclidean distance; loss = sum over pairs / B.

Sharding: data-parallel over batch — 4 batches (32 (b,c) pairs) per core.
Each core reduces its pairs to per-column partial sums; the host finishes
the few-hundred-flop scalar reconstruction (coords, sqrt, sum).

Layout per pair: the 65536-element heatmap as [128 partitions x 512 free],
flat index = p*512 + jj, so h = 2p + (jj>=256), w = jj % 256.

Flipped matmuls: e = exp(x) (bf16) is the STATIONARY operand in 128-col
chunks; the moving operand is a 32-col one-hot (ones / (2p+1)-weighted)
selecting the pair's column, so each matmul costs only 32 moving columns
(ldweights is free).  PSUM accumulates, per chunk c, the per-column sums
colsum_c[r, i] = sum_p e_i[p, c*128+r] for pairs 0..29, plus an
a-weighted tile sum_p e_i[p,*]*(2p+1).  Pairs 30 and 31 go to separate
tiny [128, 5] PSUMs so the big tiles stop (and are copied to SBUF) two
pairs early — only the last pair's short chain sits after the final
input DMA, and the five big stats copies clear the DVE well before the
tail needs it.

Output: TWO prepared SWDGE writebacks fired with trigger_dma (skipping
the ~1.3us HWDGE launch latency): the first carries the [128, 160] bf16
stats tile (chunk colsums pairs 0-29, a-sums; ~0.2% storage rounding vs
the 2e-2 gate, half the writeback bytes) and fires right after the big
copies — its transfer hides before the tail; the second carries a tiny
[128, 12] f32 tile (pair-30/31 chunk colsums + a, plus the
integer-exact target p*/j*) and is the only DMA on the tail critical
path.

Target argmax: DVE max (top-8 per partition) + max_index per pair, then a
PE transpose and one more max/max_index across partitions; first-occurrence
semantics match jnp.argmax.  p*/j* land in columns 10:12 of the small
tile.

Schedule: all target DMAs stream first (DVE argmax + epilogue hide under
the input stream).  The post-stream critical path is the serial ACT exp
chain: a piece of c columns costs 184 + 0.836c ns of exp against a
1.456c ns transfer, so only pieces >= ~300 cols keep ACT ahead of
arrivals, while a small final piece poisons the suffix terms of the
pieces before it.  Optimum within per-pair pieces: pairs 0-21 as
doubles, 22-29 as full singles, pairs 30/31's first chunks (128c)
streamed EARLY where mid-stream slack absorbs them, ending
[s28, s29, 30b(384c), 31b(384c)] — worst suffix term exp(384c) = 505ns.
Pair 30's PSUM copy runs on the otherwise-idle ACT engine; pair 31's
(the true tail) on DVE.  Constants are generated on-chip — HBM traffic
is exactly input+target.

Post-compile patches (same machinery the framework itself can't express):
  (a) the first target DMA is hoisted to the top of the preamble block,
      before SP's Drain and barrier EVSEM — it has no preamble
      dependency, so its transfer starts ~0.7us earlier and the whole
      stream shifts left;
  (b) each trigger_dma gets engine-tick waits so it fires only after the
      copies into its writeback tile (DVE tick for both; ACT tick too
      for the second);
  (c) the first epilogue barrier's Pool gather waits wb_dma>=32, gating
      function end on both writebacks' true completion (this must sit
      BEFORE the epilogue's semaphore range-clear, which zeroes wb_dma);
  (d) the framework's serial SWDGE queue-drain checks are dropped and
      the second epilogue barrier round (after the clear) is deleted:
      barrier 1 — completion-gated via (c) — already has every engine
      idle before the clear, and NEFF completion still requires Pool's
      queue (ending with the clear) to drain, so nothing races the
      clear or the next execution;
  (e) the signals_writable ordering sem (only needed so tile doesn't
      hoist the dep-free second trigger) is erased from the runtime
      program — its DMA-path updates would otherwise fire 900ns late
      and stall the epilogue's engine-lane checks.
"""

import numpy as np

B, C, H, W = 32, 8, 256, 256
N_CORES = 8
PAIRS = (B // N_CORES) * C          # 32 (b,c) pairs per core
P = 128                             # SBUF partitions
F = (H * W) // P                    # 512 free elements per partition
CH = 128                            # stationary chunk width (PE limit)
GT = 2                              # pairs per target DMA group
NGT = PAIRS // GT
NBIG = 30                           # pairs accumulated in the big PSUM tiles
NCN = 160                           # big writeback cols: 4*32 e + 32 a (bf16)
NCN2 = 12                           # small wb cols: 2 pairs x 5 + p* + j* (f32)
N_DOUBLE = 11                       # input doubles: pairs 0..21
SINGLES = list(range(22, 30))       # input singles: pairs 22..29
# tail order: [s22..s27, 30a(128c), 31a(128c), s28, s29, 30b(384c), 31b(384c)]

_nc_cache = None


def _build_nc():
    import concourse.mybir as mybir
    import concourse.tile as tile
    from concourse import bacc

    f32 = mybir.dt.float32
    bf16 = mybir.dt.bfloat16
    u32 = mybir.dt.uint32
    i32 = mybir.dt.int32
    AF = mybir.ActivationFunctionType
    ALU = mybir.AluOpType
    AX = mybir.AxisListType

    nc = bacc.Bacc("TRN2", target_bir_lowering=False, debug=False,
                   num_devices=N_CORES)

    inp = nc.dram_tensor("input", [PAIRS, P, F], f32, kind="ExternalInput").ap()
    tgt = nc.dram_tensor("target", [PAIRS, P, F], f32, kind="ExternalInput").ap()
    out_d = nc.dram_tensor("out", [P, NCN], bf16, kind="ExternalOutput").ap()
    out2_d = nc.dram_tensor("out2", [P, NCN2], f32, kind="ExternalOutput").ap()

    wb_sem = nc.alloc_semaphore("wb_dma")

    with tile.TileContext(nc) as tc:
        with (
            tc.tile_pool(name="const", bufs=1) as constp,
            tc.tile_pool(name="stats", bufs=1) as statsp,
            tc.tile_pool(name="inwd", bufs=5) as inwdp,
            tc.tile_pool(name="inws", bufs=10) as inwsp,
            tc.tile_pool(name="tgw", bufs=12) as tgwp,
            tc.tile_pool(name="ewd", bufs=5) as ewdp,
            tc.tile_pool(name="ews", bufs=10) as ewsp,
            tc.tile_pool(name="psum", bufs=1, space="PSUM") as psp,
        ):
            # ---- on-chip constants (GPSIMD; no HBM traffic) ----
            # one-hot bank: column PAIRS-1 is all ones; slice
            # [PAIRS-1-i : 2*PAIRS-1-i] puts the ones at local column i so
            # pair i's colsums land in PSUM column i of the moving operand
            oh = constp.tile([P, 2 * PAIRS - 1], bf16)
            nc.gpsimd.memset(oh[:], 0.0)
            nc.gpsimd.memset(oh[:, PAIRS - 1:PAIRS], 1.0)
            # same sliding bank but with column PAIRS-1 = (2p+1)
            oha = constp.tile([P, 2 * PAIRS - 1], bf16)
            nc.gpsimd.memset(oha[:], 0.0)
            nc.gpsimd.iota(oha[:, PAIRS - 1:PAIRS], pattern=[[0, 1]], base=1,
                           channel_multiplier=2,
                           allow_small_or_imprecise_dtypes=True)
            # pair-30/31 moving operands: per chunk c, col c = 1, col 4 = 2p+1
            movs = constp.tile([P, 20], bf16)
            nc.gpsimd.memset(movs[:], 0.0)
            for c in range(4):
                nc.gpsimd.memset(movs[:, 5 * c + c:5 * c + c + 1], 1.0)
            nc.gpsimd.iota(
                movs[:].rearrange("p (g f) -> p g f", g=4)[:, :, 4:5],
                pattern=[[0, 4], [0, 1]], base=1, channel_multiplier=2,
                allow_small_or_imprecise_dtypes=True)
            ident = constp.tile([P, P], f32)
            nc.gpsimd.memset(ident[:], 1.0)
            nc.gpsimd.affine_select(ident[:], ident[:], pattern=[[1, P]],
                                    compare_op=ALU.is_equal, fill=0.0,
                                    base=0, channel_multiplier=-1)
            iota_row = constp.tile([PAIRS, P], f32)   # each row: 0..127
            nc.gpsimd.iota(iota_row[:], pattern=[[1, P]], base=0,
                           channel_multiplier=0,
                           allow_small_or_imprecise_dtypes=True)
            ctx_idxs = constp.tile([P, 1], i32)       # writeback ctx index 0
            nc.gpsimd.memset(ctx_idxs[:], 0)

            # the output stats tiles; zeroed so unwritten lanes are defined.
            # The big tile is bf16: colsums only need ~0.2% relative accuracy
            # (harness gate is 2e-2) and 16-bit halves both the DVE copy time
            # and the writeback bytes.  Integer-exact p*/j* live in the f32
            # small tile instead.
            wbt = statsp.tile([P, NCN], bf16)
            nc.gpsimd.memset(wbt[:], 0.0)
            wbt2 = statsp.tile([P, NCN2], f32)
            nc.gpsimd.memset(wbt2[:], 0.0)

            # prepare both writebacks EARLY so SWDGE desc-gen and the Q7
            # library reload run on the idle Pool engine before the streams
            # get going.  Descriptors only encode SBUF addresses; the DMAs
            # read the tiles when their triggers fire.  BOTH preps come
            # before the triggers: Pool SEQ is in-order, and trig0 parks on
            # its (patched) DVE wait until ~49us — prep1's ~1us Q7 descgen
            # must not queue behind that park.  count=1 pops the ring FIFO
            # in prep order: trig0 fires the big tile, trig1 the small one.
            nc.gpsimd.kv_writeback(
                out_d.rearrange("(b p) (o n) -> b p o n", b=1, o=1),
                wbt[:].rearrange("p (o b n) -> p o b n", o=1, b=1),
                ctx_idxs[:], prepare_only=True, sem=wb_sem)
            nc.gpsimd.kv_writeback(
                out2_d.rearrange("(b p) (o n) -> b p o n", b=1, o=1),
                wbt2[:].rearrange("p (o b n) -> p o b n", o=1, b=1),
                ctx_idxs[:], prepare_only=True, sem=wb_sem)
            # WAW on a dummy tile pins trig1 after trig0 at tile-scheduling
            # time (the real firing constraints are the post-compile DVE
            # waits, which tile can't see — without the WAW it hoists the
            # dep-free trig1 to the front of the Pool stream, where its park
            # would block everything behind it)
            trig_order = constp.tile([1, 1], f32)
            trig0 = nc.gpsimd.trigger_dma(count=1,
                                          signals_writable=[trig_order[:]])
            trig1 = nc.gpsimd.trigger_dma(count=1,
                                          signals_writable=[trig_order[:]])

            pmax8 = statsp.tile([P, 8 * PAIRS], f32)   # per-pair top-8 of target
            pidx8 = statsp.tile([P, 8 * PAIRS], u32)   # ... and their indices

            # PSUM accumulators: per chunk c, colsum_c[r, i] for pairs 0..29;
            # one (2p+1)-weighted tile; pairs 30/31 in their own [128, 5].
            psum_e = [psp.tile([P, PAIRS], f32, name=f"psum_e{c}")
                      for c in range(4)]
            psum_a = psp.tile([P, PAIRS], f32)
            # separate tiles: tile-granular dependency tracking would
            # otherwise chain pair-30's copy behind pair-31's matmuls
            psum_s = {30: psp.tile([P, 5], f32, name="psum_s30"),
                      31: psp.tile([P, 5], f32, name="psum_s31")}

            # ---- phase 1: stream target, per-pair per-partition argmax ----
            for g in range(NGT):
                tt = tgwp.tile([P, GT * F], f32)
                nc.sync.dma_start(
                    tt[:].rearrange("p (n m) -> p n m", n=GT),
                    tgt[g * GT:(g + 1) * GT].rearrange("n p m -> p n m"))
                for k in range(GT):
                    i = g * GT + k
                    sl_tg = tt[:, k * F:(k + 1) * F]
                    nc.vector.max(pmax8[:, 8 * i:8 * i + 8], sl_tg)
                    nc.vector.max_index(pidx8[:, 8 * i:8 * i + 8],
                                        pmax8[:, 8 * i:8 * i + 8], sl_tg)

            # ---- phase 2: target epilogue (runs while input streams) ----
            pmaxc = statsp.tile([P, PAIRS], f32)
            pidxf = statsp.tile([P, PAIRS], f32)
            nc.vector.tensor_copy(
                pmaxc[:].rearrange("p (n o) -> p n o", o=1),
                pmax8[:].rearrange("p (n e) -> p n e", e=8)[:, :, 0:1])
            nc.vector.tensor_copy(
                pidxf[:].rearrange("p (n o) -> p n o", o=1),
                pidx8[:].rearrange("p (n e) -> p n e", e=8)[:, :, 0:1])

            # one shared transpose PSUM tile (sequential use) keeps the
            # bank budget at 8 so pairs 30/31 get their own tiles below
            ps_t = psp.tile([PAIRS, P], f32, name="ps_t")
            nc.tensor.transpose(ps_t[:], pmaxc[:], ident[:])
            pmaxT = statsp.tile([PAIRS, P], f32)
            nc.vector.tensor_copy(pmaxT[:], ps_t[:])
            nc.tensor.transpose(ps_t[:], pidxf[:], ident[:])
            pidxT = statsp.tile([PAIRS, P], f32)
            nc.vector.tensor_copy(pidxT[:], ps_t[:])

            g8 = statsp.tile([PAIRS, 8], f32)
            nc.vector.max(g8[:], pmaxT[:])
            gp8 = statsp.tile([PAIRS, 8], u32)
            nc.vector.max_index(gp8[:], g8[:], pmaxT[:])
            pst = statsp.tile([PAIRS, 1], f32)
            nc.vector.tensor_copy(pst[:], gp8[:, 0:1])           # p*
            nc.vector.tensor_copy(wbt2[0:PAIRS, 10:11], pst[:])

            # j* = pidxT[i, p*_i] via equality mask + fused mul-reduce
            mask = statsp.tile([PAIRS, P], f32)
            nc.vector.tensor_scalar(mask[:], iota_row[:], pst[:], None,
                                    op0=ALU.is_equal)
            mscr = statsp.tile([PAIRS, P], f32)
            nc.vector.tensor_mul(mscr[:], mask[:], pidxT[:])
            nc.vector.reduce_sum(wbt2[0:PAIRS, 11:12], mscr[:], axis=AX.X)

            # ---- phase 3: stream input, exp + flipped one-hot matmuls ----
            def pair_matmuls(e_ap, i, k):
                ohs = oh[:, PAIRS - 1 - i:2 * PAIRS - 1 - i]
                ohas = oha[:, PAIRS - 1 - i:2 * PAIRS - 1 - i]
                st, sp = (i == 0), (i == NBIG - 1)
                for c in range(4):
                    ec = e_ap[:, k * F + c * CH:k * F + (c + 1) * CH]
                    nc.tensor.matmul(psum_e[c][:], ec, ohs,
                                     start=st, stop=sp)
                    nc.tensor.matmul(psum_a[:], ec, ohas,
                                     start=(st and c == 0),
                                     stop=(sp and c == 3))

            for g in range(N_DOUBLE):
                i0 = g * 2
                ti = inwdp.tile([P, 2 * F], f32, tag="tid")
                nc.sync.dma_start(
                    ti[:].rearrange("p (n m) -> p n m", n=2),
                    inp[i0:i0 + 2].rearrange("n p m -> p n m"))
                # bf16 e: full-rate PE moving/stationary; PSUM stays f32
                e = ewdp.tile([P, 2 * F], bf16, tag="ed")
                nc.scalar.activation(e[:], ti[:], AF.Exp)
                for k in range(2):
                    pair_matmuls(e, i0 + k, k)

            def small_pair_matmuls(e_ap, i, chunks):
                for c in chunks:
                    ec = e_ap[:, c * CH:(c + 1) * CH]
                    nc.tensor.matmul(psum_s[i], ec,
                                     movs[:, 5 * c:5 * c + 5],
                                     start=(c == 0), stop=(c == 3))

            def single(i):
                ti = inwsp.tile([P, F], f32, tag="tis")
                nc.sync.dma_start(
                    ti[:].rearrange("p (n m) -> p n m", n=1),
                    inp[i:i + 1].rearrange("n p m -> p n m"))
                e = ewsp.tile([P, F], bf16, tag="es")
                nc.scalar.activation(e[:], ti[:], AF.Exp)
                pair_matmuls(e, i, 0)

            # tail: the ACT exp chain is the post-stream critical path.
            # Each piece is ready at its own DMA end + 900ns; exp(c cols) =
            # 184 + 0.836c vs transfer 1.456c, so pieces >= ~300 cols keep
            # ACT ahead of arrivals while small pieces poison the suffix
            # terms via their short transfers.  Streaming pairs 30/31's
            # first chunks (128c each) EARLY — where mid-stream slack
            # absorbs them — and ending [s28, s29, 30b(384c), 31b(384c)]
            # makes the worst suffix term exp(384c) = 505ns (vs 612 all-full
            # pairs, ~1770 naive fine splitting).
            tis = {i: inwsp.tile([P, F], f32, tag="tis", name=f"ti{i}")
                   for i in (30, 31)}
            es = {i: ewsp.tile([P, F], bf16, tag="es", name=f"e{i}")
                  for i in (30, 31)}

            def tail_piece(i, sl, chunks):
                nc.sync.dma_start(
                    tis[i][:, sl].rearrange("p (n m) -> p n m", n=1),
                    inp[i:i + 1, :, sl].rearrange("n p m -> p n m"))
                nc.scalar.activation(es[i][:, sl], tis[i][:, sl], AF.Exp)
                small_pair_matmuls(es[i], i, chunks)

            for i in SINGLES[:-2]:
                single(i)
            tail_piece(30, slice(0, 128), range(0, 1))
            tail_piece(31, slice(0, 128), range(0, 1))
            for i in SINGLES[-2:]:
                single(i)
            tail_piece(30, slice(128, 512), range(1, 4))
            tail_piece(31, slice(128, 512), range(1, 4))

            # ---- phase 4: PSUM -> stats tiles ----
            # big tiles stopped at pair 29 — these five copies run well
            # before the tail and are what trigger0 waits for
            for c in range(4):
                nc.vector.tensor_copy(wbt[:, 32 * c:32 * (c + 1)],
                                      psum_e[c][:])
            nc.vector.tensor_copy(wbt[:, 128:160], psum_a[:])
            # pair 30's copy runs on the otherwise-idle ACT engine (its
            # data is ready while DVE is still draining the big copies);
            # pair 31's copy — the true tail — stays on DVE.  trigger1 is
            # patched to wait BOTH engines' ticks.
            nc.scalar.copy(wbt2[:, 0:5], psum_s[30])
            nc.vector.tensor_copy(wbt2[:, 5:10], psum_s[31])

    # --- post sem-assignment patches -------------------------------------
    import bass_rust
    fn = nc.m.functions[0]
    body = fn.blocks[1]

    # (a) hoist the first target DMA into the preamble block, before SP's
    #     barrier EVSEM: it has no dependency on the preamble (fresh SBUF
    #     tile, static DGE table, sems cleared by the previous run's
    #     epilogue), and SP's per-engine order is unchanged, so every
    #     lane-sem tick still matches.  The transfer starts ~0.7us earlier.
    first_dma = None
    for ins in body.instructions:
        if type(ins).__name__ == "InstDMACopy":
            first_dma = ins
            break
    assert first_dma is not None and \
        first_dma.engine == mybir.EngineType.SP, first_dma
    assert not (first_dma.sync_info and first_dma.sync_info.on_wait), \
        "first DMA must be wait-free to hoist pre-barrier"
    entry = fn.blocks[0]
    sp_first_idx = None
    for idx, ins in enumerate(entry.instructions):
        if getattr(ins, "engine", None) == mybir.EngineType.SP:
            sp_first_idx = idx
            break
    assert sp_first_idx is not None
    body.instructions.remove(first_dma)
    entry.instructions.insert(sp_first_idx, first_dma)

    # (b) each trigger fires only after the copies into its tile.  trig0
    #     waits the DVE tick of the last big-stats copy (2nd-from-last DVE
    #     copy... the p31 copy is last); trig1 waits the last DVE copy
    #     (pair 31) AND the ACT tick of the pair-30 scalar copy (the last
    #     ACT body instruction).
    dve = act = None
    dtick = atick = 0
    copy_ticks = []
    act_last_tick = 0
    for blk in fn.blocks:
        for ins in blk.instructions:
            si = ins.sync_info
            if si is None:
                continue
            for u in si.on_update:
                if u.ant_name and u.ant_name.startswith("DVE_"):
                    dve = (u.ant_name, u.id)
                    dtick += u.update_value
                if u.ant_name and u.ant_name.startswith("Activation_"):
                    act = (u.ant_name, u.id)
                    atick += u.update_value
            eng = getattr(ins, "engine", None)
            if (type(ins).__name__ == "InstTensorCopy"
                    and eng == mybir.EngineType.DVE):
                copy_ticks.append(dtick)
            if (eng == mybir.EngineType.Activation
                    and blk is body):
                act_last_tick = atick
    assert dve and act and len(copy_ticks) >= 7, (dve, act, len(copy_ticks))
    trig_waits = ((trig0, ((dve, copy_ticks[-2]),)),
                  (trig1, ((dve, copy_ticks[-1]), (act, act_last_tick))))
    for trig, waits in trig_waits:
        if trig.ins.sync_info is None:
            trig.ins.sync_info = bass_rust.SyncInfo(on_wait=[], on_update=[])
        # drop the signals_writable ordering sem wait: it was only needed so
        # tile wouldn't hoist the dep-free trigger, but at runtime its update
        # fires a full SEM_PROP_DMA_OVERHEAD (900ns) after trig0's transfer,
        # which would serialize trig1 behind it.  The tick waits enforce the
        # true firing order.
        trig.ins.sync_info.on_wait = [
            w for w in trig.ins.sync_info.on_wait
            if not (w.ant_name and w.ant_name.startswith("Pool_sequencer"))]
        for (name, sid), t in waits:
            trig.ins.sync_info.on_wait.append(bass_rust.SyncWait(
                sync_type="semaphore", id=sid, ant_name=name,
                wait_mode="sem-ge-imm", wait_value=t))

    # (c) gate function end on both writebacks' completion: the first
    #     epilogue barrier's Pool gather (the EVSEM waiting on the barrier
    #     gather sem) also waits wb_dma>=32.  This sits BEFORE the
    #     epilogue's semaphore range-clear, which zeroes wb_dma — gating
    #     anything later would deadlock.  The framework's own queue-drain
    #     waits don't cover this: the SWDGE lane sem is bumped at prep
    #     (descriptor-gen) time on HW, so they pass before the data lands.
    epi = fn.blocks[2]
    gate = None
    for ins in epi.instructions:
        si = ins.sync_info
        if (type(ins).__name__ == "InstEventSemaphore"
                and ins.engine == mybir.EngineType.Pool
                and si and any(w.ant_name and "gather" in w.ant_name
                               for w in si.on_wait)):
            gate = ins
            break
    assert gate is not None, "epilogue Pool barrier gather not found"
    assert len(gate.sync_info.on_wait) < 2
    wb = None
    for blk in fn.blocks:
        for ins in blk.instructions:
            si = ins.sync_info
            if si is None:
                continue
            for u in si.on_update:
                if u.ant_name == "wb_dma":
                    wb = (u.ant_name, u.id)
    assert wb is not None
    gate.sync_info.on_wait.append(bass_rust.SyncWait(
        sync_type="semaphore", id=wb[1], ant_name=wb[0],
        wait_mode="sem-ge-imm", wait_value=32))

    # (d) the framework's SWDGE queue-drain checks (SP EVSEMs waiting
    #     DMASW*>=32, i.e. prep + completion) sit serially on SP's path
    #     AFTER the last writeback's +900ns completion prop.  Drop them:
    #     the barrier release is already gated on wb_dma>=32 via (c) —
    #     the same DMA-completion signals — so the queue-drained-before-
    #     clear protection holds while SP's checks pass early.
    for ins in epi.instructions:
        si = ins.sync_info
        if (type(ins).__name__ == "InstEventSemaphore"
                and ins.engine == mybir.EngineType.SP
                and si and any(w.ant_name and w.ant_name.startswith("DMASW")
                               for w in si.on_wait)):
            si.on_wait = [w for w in si.on_wait
                          if not (w.ant_name
                                  and w.ant_name.startswith("DMASW"))]

    # (d') drop the second epilogue barrier: barrier 1 (gated on wb_dma via
    #     (c)) already has every engine idle before Pool's sem range-clear,
    #     and NEFF completion requires Pool's queue (ending with the clear)
    #     to drain, so nothing races the clear or the next execution.  The
    #     barrier-2 sems are self-contained, so deleting the whole round
    #     leaves no dangling waits.
    clear_idx = None
    for idx, ins in enumerate(epi.instructions):
        if (type(ins).__name__ == "InstISA"
                and ins.engine == mybir.EngineType.Pool):
            clear_idx = idx
    assert clear_idx is not None, "epilogue sem range-clear not found"
    del epi.instructions[clear_idx + 1:]

    # (e) erase every runtime trace of the signals_writable ordering sem:
    #     it only existed for tile scheduling, but its DMA-path updates fire
    #     +900ns after each trigger's transfer, and the framework's epilogue
    #     waits on it — which would put trig1+900 on the critical path all
    #     over again.  Removing both updates and waits is sound: nothing
    #     else reads the sem, and writeback completion is still gated by
    #     wb_dma via (c).
    for blk in fn.blocks:
        for ins in blk.instructions:
            si = ins.sync_info
            if si is None:
                continue
            if any(u.ant_name and u.ant_name.startswith("Pool_sequencer")
                   for u in si.on_update):
                si.on_update = [
                    u for u in si.on_update
                    if not (u.ant_name
                            and u.ant_name.startswith("Pool_sequencer"))]
            if any(w.ant_name and w.ant_name.startswith("Pool_sequencer")
                   for w in si.on_wait):
                si.on_wait = [
                    w for w in si.on_wait
                    if not (w.ant_name
                            and w.ant_name.startswith("Pool_sequencer"))]

    nc.compile()
    return nc


def _get_nc():
    global _nc_cache
    if _nc_cache is None:
        _nc_cache = _build_nc()
    return _nc_cache


def _in_maps(input, target):
    input = np.ascontiguousarray(np.asarray(input, dtype=np.float32))
    target = np.ascontiguousarray(np.asarray(target, dtype=np.float32))
    bpc = B // N_CORES
    maps = []
    for c in range(N_CORES):
        maps.append({
            "input": input[c * bpc:(c + 1) * bpc].reshape(PAIRS, P, F),
            "target": target[c * bpc:(c + 1) * bpc].reshape(PAIRS, P, F),
        })
    return maps


def _finish(out, out2):
    """Host-side reconstruction from the stats tiles (f64 math).

    out [128, 160] bf16:
      cols 32c..32c+31: colsum_c[r, i] = sum_p e_i[p, c*128+r], pairs 0..29
      cols 128..159:    ya[r, i]       = sum_p e_i[p, *] * (2p+1), pairs 0..29
    out2 [128, 12] f32:
      cols 0..3: pair-30 chunk colsums; col 4: pair-30 ya
      cols 5..8: pair-31 chunk colsums; col 9: pair-31 ya
      col 10 rows 0..31: p*; col 11 rows 0..31: j*
    """
    s = np.asarray(out).astype(np.float64).reshape(P, NCN)
    s2 = np.asarray(out2, dtype=np.float64).reshape(P, NCN2)
    cols = np.empty((512, PAIRS))                       # [jj, pair]
    for c in range(4):
        cols[c * 128:(c + 1) * 128, :] = s[:, 32 * c:32 * c + 32]
        cols[c * 128:(c + 1) * 128, 30] = s2[:, c]
        cols[c * 128:(c + 1) * 128, 31] = s2[:, 5 + c]
    ya = s[:, 128:160].sum(axis=0)
    ya[30] = s2[:, 4].sum()
    ya[31] = s2[:, 9].sum()
    jj = np.arange(512)
    xw = (jj % W) + 1.0
    hi = (jj >= W).astype(np.float64)
    S = cols.sum(axis=0)
    X = (cols * xw[:, None]).sum(axis=0)
    Yb = (cols * hi[:, None]).sum(axis=0)
    pred_x = X / S / W
    pred_y = (ya + Yb) / S / H
    pstar = s2[0:PAIRS, 10]
    jstar = s2[0:PAIRS, 11]
    rr = (jstar >= W).astype(np.float64)
    wcoord = jstar - W * rr
    hcoord = 2.0 * pstar + rr
    tx = (wcoord + 1.0) / W
    ty = (hcoord + 1.0) / H
    return float(np.sqrt((tx - pred_x) ** 2 + (ty - pred_y) ** 2).sum())


def run(input, target, trace=False):
    """Run on hardware; returns (loss, BassKernelResults)."""
    from concourse.bass_utils import run_bass_kernel_spmd
    nc = _get_nc()
    res = run_bass_kernel_spmd(nc, _in_maps(input, target),
                               list(range(N_CORES)), trace=trace)
    total = sum(_finish(r["out"], r["out2"]) for r in res.results)
    return np.float32(total / B), res


def kernel(**inputs):
    loss, _ = run(inputs["input"], inputs["target"])
    return np.asarray(loss, dtype=np.float32)
